# revision 23
# baseline (speedup 1.0000x reference)
"""TRN2 Bass kernel for nn_GAT_34282428956965 (3-layer GAT on lifted node+edge graph).

Key identity: the reference's einsum('bnnk,bnkd->bnkd') only uses the DIAGONAL
of the row-softmax, so out[i] = p_i * Xs[i] with
    p_i = exp(lrelu(s_i+t_i)) / sum_{j in nbr(i)} exp(lrelu(s_i+t_j)),
and the adjacency VALUES only matter as a nonzero mask.  The graph has ~41K
nonzeros vs 81M dense entries, so denominators are computed sparsely with one
GPSIMD ap_gather per layer plus regular PE/ACT/DVE work.
exp(lrelu(u)) = max(exp(u), exp(0.2u)) avoids the unsupported Lrelu table.

Layouts:
  x_T [64, 9000]    activations transposed (features on partitions), col n = T-row n
                    (cols 0..1000 nodes, col 1000+8i+m = edge (i,m))
  TABLE [128, 9104] t_n broadcast to all partitions (cols 9000+ = -1e30 dummies)
  half-band h in [0,8): partitions [16h, 16h+16) = gpsimd core h owns
                    edges [1000h, 1000h+1000) and nodes [125h, 125h+125)
  U [128, NU]       exp(lrelu(.)) args batched: [dst 1000 | in-grid 125*K16 |
                    self 1000 | src 1000 | out 1000 | nself 125]
  matmuls run as float32r (4x PE rate, ~1e-4 rel err)
"""
import numpy as np

import concourse.bass as bass
import concourse.mybir as mybir
import concourse.tile as tile
from concourse import bacc
from concourse.bass_utils import run_bass_kernel_spmd

dt = mybir.dt
AF = mybir.ActivationFunctionType
OP = mybir.AluOpType

N, M, H, FIN = 1000, 8, 64, 32
E, T = N * M, N + N * M          # 8000, 9000
NEG = 0.2
NEGBIG = -1e30
DUMMY = T                        # table col holding -1e30
TPAD = 9104                      # table cols (mult of 16, room for dummies)
NSG = 4                          # 32-partition supergroups (PE col-position grid)
ESG, NSGN = E // NSG, N // NSG   # 2000 edges, 250 nodes per supergroup
EHB, NHB = ESG // 2, NSGN // 2   # 1000 edges, 125 nodes per 16-row half-band
CH = 512                         # psum matmul chunk (one fp32 bank)
FAST = True                      # float32r matmuls


def _chunks(lo, hi, step=CH):
    return [(a, min(a + step, hi)) for a in range(lo, hi, step)]


def build(K16: int, stage: int = 99, repeat: int = 1):
    GIN = NHB * K16              # in-grid cols per half-band (per core)
    NIG = EHB + GIN              # gathered idxs per gpsimd core
    assert NIG % 16 == 0
    O_DST, O_IN = 0, EHB
    O_SELF = O_IN + GIN
    O_SRC = O_SELF + EHB
    O_OUT = O_SRC + EHB
    O_NSELF = O_OUT + EHB
    NU = O_NSELF + NHB

    nc = bacc.Bacc("TRN2", target_bir_lowering=False, debug=False)

    def mm(out, lhsT, rhs, **kw):
        nc.tensor.matmul(out, lhsT, rhs, **kw)

    node_in_d = nc.dram_tensor("node_in_T", [FIN, N], dt.float32, kind="ExternalInput")
    edge_d = nc.dram_tensor("edge_T", [FIN, E], dt.float32, kind="ExternalInput")
    idx_d = nc.dram_tensor("idxT", [128, NIG // 16], dt.int16, kind="ExternalInput")
    # per layer l at col 320*l: [W(64) | WS128(128) | WT128(128)]
    wpack_d = nc.dram_tensor("wpack", [H, 960], dt.float32, kind="ExternalInput")
    emb_d = nc.dram_tensor("embpack", [FIN, 2 * H], dt.float32, kind="ExternalInput")
    bias_d = nc.dram_tensor("biasT", [H, 2], dt.float32, kind="ExternalInput")
    sel_d = nc.dram_tensor("selpack", [128, 8 * H], dt.float32, kind="ExternalInput")
    mask_d = nc.dram_tensor("maskP", [128, 9], dt.uint8, kind="ExternalInput")
    out_d = nc.dram_tensor("out_T", [H, N], dt.float32, kind="ExternalOutput")

    with tile.TileContext(nc) as tc:
        with (
            tc.tile_pool(name="const", bufs=1) as cpool,
            tc.tile_pool(name="state", bufs=1) as spool,
            tc.tile_pool(name="work", bufs=3) as wpool,
            tc.tile_pool(name="mm", bufs=4, space="PSUM") as mpool,
        ):
            wdt = dt.float32r if FAST else dt.float32
            wpack = cpool.tile([H, 960], wdt)
            emb = cpool.tile([FIN, 2 * H], wdt)
            biasc = cpool.tile([H, 2], dt.float32)
            idxT = cpool.tile([128, NIG // 16], dt.int16)
            selp = cpool.tile([128, 8 * H], wdt)
            maskc = cpool.tile([128, 9], dt.uint8)
            nc.gpsimd.dma_start(wpack[:], wpack_d[:])
            nc.gpsimd.dma_start(emb[:], emb_d[:])
            nc.sync.dma_start(biasc[:], bias_d[:])
            nc.sync.dma_start(idxT[:], idx_d[:])
            nc.gpsimd.dma_start(selp[:], sel_d[:])
            nc.sync.dma_start(maskc[:], mask_d[:])

            x_T = spool.tile([H, T], wdt)

            for _rep in range(repeat):
                # ---- embeddings (inputs alias big per-layer slots, dead after) ----
                node_in = spool.tile([FIN, N], wdt, tag="REC")
                edge_in = spool.tile([FIN, E], wdt, tag="U")
                nc.gpsimd.dma_start(node_in[:], node_in_d[:])
                nc.gpsimd.dma_start(edge_in[:], edge_d[:])
                for lo, hi in _chunks(0, N):
                    ps = mpool.tile([128, CH], dt.float32, tag="mm")
                    mm(ps[:H, : hi - lo], emb[:, 0:H], node_in[:, lo:hi])
                    nc.vector.tensor_scalar_add(
                        x_T[:, lo:hi], ps[:H, : hi - lo], biasc[:, 0:1]
                    )
                for lo, hi in _chunks(0, E):
                    ps = mpool.tile([128, CH], dt.float32, tag="mm")
                    mm(ps[:H, : hi - lo], emb[:, H:], edge_in[:, lo:hi])
                    nc.vector.tensor_scalar_add(
                        x_T[:, N + lo : N + hi], ps[:H, : hi - lo], biasc[:, 1:2]
                    )
                if stage == 0:
                    nc.gpsimd.dma_start(out_d[:], x_T[:, :N])
                    continue

                TABLE = spool.tile([128, TPAD], dt.float32)
                SE2 = spool.tile([128, EHB], dt.float32)
                TE2 = spool.tile([128, EHB], dt.float32)
                SN2 = spool.tile([128, NHB], dt.float32)
                TN2 = spool.tile([128, NHB], dt.float32)
                U = spool.tile([128, NU], dt.float32, tag="U")
                INS = spool.tile([128, NHB], dt.float32)
                OUTS = spool.tile([128, NHB], dt.float32)
                REC = spool.tile([128, EHB], dt.float32, tag="REC")
                PEDGE = spool.tile([128, EHB], wdt)
                PNODE = spool.tile([128, NHB + 1], wdt)
                EXCH = 2400
                scr = spool.tile([128, EXCH], dt.float32)

                for l in range(3):
                    wc = 320 * l
                    W_l = wpack[:, wc : wc + 64]
                    WS128 = wpack[:, wc + 64 : wc + 192]
                    WT128 = wpack[:, wc + 192 : wc + 320]

                    # t broadcast table (PE) + dummy cols
                    for lo, hi in _chunks(0, T):
                        ps = mpool.tile([128, CH], dt.float32, tag="mm")
                        mm(ps[:, : hi - lo], WT128, x_T[:, lo:hi])
                        nc.vector.tensor_copy(TABLE[:, lo:hi], ps[:, : hi - lo])
                    nc.vector.memset(TABLE[:, T:TPAD], NEGBIG)
                    if stage == 1 and l == 0:
                        nc.sync.dma_start(out_d[:], TABLE[:H, :N])
                        break

                    # half-band fields via full-array f32r matmuls + mask-merge
                    # (f32r forbids PE tile_position; rows [16h,16h+16) want cols
                    #  of half-band h, so merge 8 broadcast matmuls per field)
                    for FLD, WX, span in (
                        (SE2, WS128, EHB), (TE2, WT128, EHB),
                        (SN2, WS128, NHB), (TN2, WT128, NHB),
                    ):
                        base = N if span == EHB else 0
                        for lo, hi in _chunks(0, span):
                            w2 = (hi - lo) + (hi - lo) % 2   # f32r needs even F
                            for h in range(8):
                                ph = mpool.tile([128, CH], dt.float32, tag="mm")
                                mm(
                                    ph[:, :w2], WX,
                                    x_T[:, base + span * h + lo : base + span * h + lo + w2],
                                )
                                if h == 0:
                                    nc.vector.tensor_copy(
                                        FLD[:, lo:hi], ph[:, : hi - lo]
                                    )
                                else:
                                    nc.vector.copy_predicated(
                                        FLD[:, lo:hi],
                                        maskc[:, 1 + h : 2 + h].to_broadcast(
                                            [128, hi - lo]
                                        ),
                                        ph[:, : hi - lo],
                                    )
                    if stage == 2 and l == 0:
                        nc.sync.dma_start(out_d[:], SE2[:H, :N])
                        break

                    # gather t[dstmod] (1000) + t'[in_slot] (125*K16) per core
                    nc.gpsimd.ap_gather(
                        out_ap=U[:, O_DST : O_IN + GIN],
                        in_ap=TABLE[:],
                        idxs_ap=idxT[:],
                        channels=128,
                        num_elems=TPAD,
                        d=1,
                        num_idxs=NIG,
                    )
                    if stage == 3 and l == 0:
                        nc.sync.dma_start(out_d[:], U[:H, :N])
                        break

                    # U segments: u = s_local + t_other
                    nc.vector.tensor_tensor(
                        U[:, O_DST:O_IN], U[:, O_DST:O_IN], SE2[:], op=OP.add
                    )
                    uin = U[:, O_IN:O_SELF].rearrange("a (n k) -> a n k", k=K16)
                    nc.vector.tensor_tensor(
                        uin, uin, SN2[:].to_broadcast([128, NHB, K16]), op=OP.add
                    )
                    nc.vector.tensor_tensor(
                        U[:, O_SELF:O_SRC], SE2[:], TE2[:], op=OP.add
                    )
                    nc.vector.tensor_tensor(
                        U[:, O_SRC:O_OUT].rearrange("a (n m) -> a n m", m=M),
                        SE2[:].rearrange("a (n m) -> a n m", m=M),
                        TN2[:].to_broadcast([128, NHB, M]),
                        op=OP.add,
                    )
                    nc.vector.tensor_tensor(
                        U[:, O_OUT:O_NSELF].rearrange("a (n m) -> a n m", m=M),
                        TE2[:].rearrange("a (n m) -> a n m", m=M),
                        SN2[:].to_broadcast([128, NHB, M]),
                        op=OP.add,
                    )
                    nc.vector.tensor_tensor(
                        U[:, O_NSELF:NU], SN2[:], TN2[:], op=OP.add
                    )

                    # W = exp(lrelu(U)) = max(exp(U), exp(0.2 U)), in place
                    for lo, hi in _chunks(0, NU, EXCH):
                        w = hi - lo
                        nc.scalar.activation(scr[:, :w], U[:, lo:hi], AF.Exp, scale=NEG)
                        nc.scalar.activation(U[:, lo:hi], U[:, lo:hi], AF.Exp)
                        nc.vector.tensor_tensor(
                            U[:, lo:hi], U[:, lo:hi], scr[:, :w], op=OP.max
                        )
                    if stage == 4 and l == 0:
                        nc.sync.dma_start(out_d[:], U[:H, :N])
                        break

                    # edge denominators -> p_edge (in place over U_self)
                    nc.vector.tensor_tensor(
                        U[:, O_DST:O_IN], U[:, O_DST:O_IN], U[:, O_SELF:O_SRC], op=OP.add
                    )
                    nc.vector.tensor_tensor(
                        U[:, O_DST:O_IN], U[:, O_DST:O_IN], U[:, O_SRC:O_OUT], op=OP.add
                    )
                    nc.vector.reciprocal(REC[:], U[:, O_DST:O_IN])
                    nc.vector.tensor_tensor(
                        PEDGE[:], U[:, O_SELF:O_SRC], REC[:], op=OP.mult
                    )
                    # node denominators -> p_node (in place over U_nself)
                    nc.vector.tensor_reduce(
                        INS[:],
                        U[:, O_IN:O_SELF].rearrange("a (n k) -> a n k", k=K16),
                        axis=mybir.AxisListType.X,
                        op=OP.add,
                    )
                    nc.vector.tensor_reduce(
                        OUTS[:],
                        U[:, O_OUT:O_NSELF].rearrange("a (n m) -> a n m", m=M),
                        axis=mybir.AxisListType.X,
                        op=OP.add,
                    )
                    nc.vector.tensor_tensor(INS[:], INS[:], OUTS[:], op=OP.add)
                    nc.vector.tensor_tensor(INS[:], INS[:], U[:, O_NSELF:NU], op=OP.add)
                    nc.vector.reciprocal(OUTS[:], INS[:])
                    nc.vector.tensor_tensor(
                        PNODE[:, :NHB], U[:, O_NSELF:NU], OUTS[:], op=OP.mult
                    )
                    if stage == 5 and l == 0:
                        nc.sync.dma_start(out_d[:], U[:H, :N])
                        break

                    # ---- apply: x_next = p * relu(Xs); layer 2: out = p*Xs, nodes only
                    def pbc_ranges(lo, hi):
                        c = lo
                        while c < hi:
                            if c < N:
                                h = c // NHB
                                e = min(hi, (h + 1) * NHB, N)
                                yield c, e, h, PNODE, (c - NHB * h, e - NHB * h)
                            else:
                                h = (c - N) // EHB
                                e = min(hi, N + (h + 1) * EHB)
                                yield c, e, h, PEDGE, (c - N - EHB * h, e - N - EHB * h)
                            c = e

                    last = l == 2
                    span = N if last else T
                    if last:
                        o_T = wpool.tile([H, N], dt.float32, tag="oT")
                    for lo, hi in _chunks(0, span):
                        pp = mpool.tile([128, CH], dt.float32, tag="mm")
                        for slo, shi, h, P2, (a, b) in pbc_ranges(lo, hi):
                            if (b - a) % 2 or (slo - lo) % 2:
                                # f32r needs even out size+offset; node pieces
                                # are 125-wide, run those as plain fp32
                                mm(
                                    pp[:H, slo - lo : shi - lo],
                                    selp[:, H * h : H * (h + 1)].bitcast(dt.float32),
                                    P2[:, a:b].bitcast(dt.float32),
                                )
                            else:
                                mm(
                                    pp[:H, slo - lo : shi - lo],
                                    selp[:, H * h : H * (h + 1)],
                                    P2[:, a:b],
                                )
                        px = mpool.tile([128, CH], dt.float32, tag="mm")
                        mm(px[:H, : hi - lo], W_l, x_T[:, lo:hi])
                        rs = wpool.tile([H, CH], dt.float32, tag="relu")
                        nc.scalar.activation(
                            rs[:, : hi - lo], px[:H, : hi - lo],
                            AF.Relu if not last else AF.Copy,
                        )
                        nc.vector.tensor_tensor(
                            (o_T if last else x_T)[:, lo:hi],
                            rs[:, : hi - lo],
                            pp[:H, : hi - lo],
                            op=OP.mult,
                        )
                    if last:
                        nc.sync.dma_start(out_d[:], o_T[:])
                    if stage == 6 and l == 0:
                        nc.gpsimd.dma_start(out_d[:], x_T[:, :N])
                        break

    nc.compile()
    return nc


_CACHE = {}


def _host_prep(node_in_fea, edge_fea, edge_fea_idx):
    node_in_fea = np.ascontiguousarray(np.asarray(node_in_fea, np.float32))
    edge_fea = np.ascontiguousarray(np.asarray(edge_fea, np.float32))
    idx = np.asarray(edge_fea_idx).astype(np.int64)

    dst = idx.reshape(E)
    src = np.repeat(np.arange(N, dtype=np.int64), M)
    selfloop = dst == src
    dstmod = np.where(selfloop, DUMMY, dst)

    # in-edge grid: for node i, table cols (N+e) of edges e with dst=i, src!=i
    eids = np.nonzero(~selfloop)[0]
    d = dst[eids]
    order = np.argsort(d, kind="stable")
    sd, se = d[order], eids[order]
    starts = np.searchsorted(sd, np.arange(N))
    ends = np.searchsorted(sd, np.arange(N) + 1)
    K_in = int((ends - starts).max())
    K16 = max(8, -(-K_in // 8) * 8)
    grid = np.full((N, K16), DUMMY, np.int64)
    ranks = np.arange(len(sd)) - starts[sd]
    grid[sd, ranks] = N + se

    # per-core (half-band) gather lists, wrapped (s p) into 16 partitions
    NIG = EHB + NHB * K16
    idxT = np.zeros((128, NIG // 16), np.int16)
    for h in range(8):
        I = np.concatenate(
            [dstmod[EHB * h : EHB * (h + 1)],
             grid[NHB * h : NHB * (h + 1)].reshape(-1)]
        ).astype(np.int16)
        idxT[16 * h : 16 * h + 16] = I.reshape(NIG // 16, 16).T
    return node_in_fea, edge_fea, idxT, K16


def make_in_map(node_in_fea, edge_fea, edge_fea_idx,
                Wn, bn, We, be,
                W1, asrc1, adst1, W2, asrc2, adst2, W3, asrc3, adst3):
    node_in_fea, edge_fea, idxT, K16 = _host_prep(
        node_in_fea, edge_fea, edge_fea_idx
    )

    f32 = lambda a: np.ascontiguousarray(np.asarray(a, np.float32))
    wpack = np.zeros((H, 960), np.float32)
    for l, (W, a_s, a_d) in enumerate(
        [(W1, asrc1, adst1), (W2, asrc2, adst2), (W3, asrc3, adst3)]
    ):
        W = f32(W)
        ws = W @ f32(a_s).reshape(H)
        wt = W @ f32(a_d).reshape(H)
        c = 320 * l
        wpack[:, c : c + 64] = W
        wpack[:, c + 64 : c + 192] = ws[:, None]
        wpack[:, c + 192 : c + 320] = wt[:, None]
    embpack = np.concatenate([f32(Wn), f32(We)], axis=1)
    biasT = np.stack([f32(bn).reshape(H), f32(be).reshape(H)], axis=1)
    selpack = np.zeros((128, 8 * H), np.float32)
    for h in range(8):
        selpack[16 * h : 16 * h + 16, H * h : H * (h + 1)] = 1.0 / 16.0
    maskP = np.zeros((128, 9), np.uint8)
    for J in range(4):
        maskP[32 * J + 16 : 32 * J + 32, 0] = 1
    for h in range(8):
        maskP[16 * h : 16 * h + 16, 1 + h] = 1

    in_map = {
        "node_in_T": np.ascontiguousarray(node_in_fea.T),
        "edge_T": np.ascontiguousarray(edge_fea.reshape(E, FIN).T),
        "idxT": idxT,
        "wpack": wpack,
        "embpack": np.ascontiguousarray(embpack),
        "biasT": np.ascontiguousarray(biasT),
        "selpack": selpack,
        "maskP": maskP,
    }
    return in_map, K16


def kernel(_stage=99, **inputs):
    in_map, K16 = make_in_map(**inputs)
    global _last_in_map
    _last_in_map = in_map
    key = (K16, _stage)
    if key not in _CACHE:
        _CACHE[key] = build(K16, _stage)
    nc = _CACHE[key]
    res = run_bass_kernel_spmd(nc, [in_map] * 8, core_ids=list(range(8)))
    return np.ascontiguousarray(res.results[0]["out_T"].T)
